# revision 24
# baseline (speedup 1.0000x reference)
"""Trainium2 Bass kernel for DendriticFullyConnected.

Math (B=128, IN=OUT=1024):
    state = sigmoid(x @ W_non.T + b_non) - 1
    syn   = x[:,None,:] * W_nmda[None,:,:]            # [B,O,I]
    clus  = 0.5*(syn[...,:-1] + syn[...,1:])          # conv [0.5,0.5]
    ca    = relu(clus.sum(-1))
    n     = 2 + state;  kd = 0.5**n;  xn = ca**n
    out   = xn/(kd+xn) + state

Key algebraic collapse: the conv+sum equals a plain dot product against
W_nmda with its first and last columns halved:
    clus.sum(-1)[b,o] = sum_i x[b,i]*Wm[o,i],  Wm = W_nmda w/ cols {0,-1} * 0.5
so the [B,O,I] tensor never exists - the whole module is two matmuls:
    z_non = x @ W_non.T + b_non ;  z_nmda = x @ Wm.T
and the Hill fraction is a sigmoid in log space:
    xn/(kd+xn) = sigmoid(n * (ln(ca) + ln 2))

Sharding: OUT split 8 ways (128 outputs/core), x replicated. Per-core HBM
traffic ~832KB; every weight byte is read exactly once across the chip.

Device-side design (v2):
- Host prepares all layouts; zero on-device transposes. Contraction index
  lands on SBUF partitions for both operands.
- z_non and z_nmda accumulate in SEPARATE PSUM groups with SEPARATE weight
  streams (W_non first). The state sigmoid chain (exp/add/recip) runs
  while the z_nmda matmuls + W_nmda DMA are still in flight.
- DMA issue split across both HWDGE rings: SP issues x + W_nmda,
  ACT issues W_non + bias (before its table load in program order).
- Matmul inputs are bf16 (f32 PSUM accumulate): memory-bound, so halving
  HBM traffic wins; measured rel-err ~2e-3 vs f32 reference (gate 2e-2).
  The b_non bias is applied as two bf16 K-rows (hi+lo split, exact to
  ~2^-17) via lhsT=ones, rhs=[b_hi],[b_lo] appended to the z_non group.
- Dummy matmuls at t~0 warm the PE HAM clock gate before the real
  matmuls arrive; memsets ride on Pool (idle) not DVE.
- Activation table pinned to natural_log_exp_and_others (exp+ln+relu) so
  the kernel pays a single ACT table load, pulled to t=0 by a dummy Ln.
"""

import numpy as np

_B, _IN, _OUT, _NC = 128, 1024, 1024, 8
_OSH = _OUT // _NC  # 128 outputs per core
_KT = _IN // 128    # 8 contraction chunks
_MMDT = "bfloat16"  # matmul input dtype

_PIN_ACT_SET = "natural_log_exp_and_others"

_state = {}


def _rearr(m):
    # [128 rows, 1024 cols] -> out[p, j*128 + r] = m[r, j*128 + p]
    # per 128-column chunk j: transpose so the contraction index is the
    # partition dim and the row index is the free dim.
    return np.ascontiguousarray(
        m.reshape(128, _KT, 128).transpose(2, 1, 0).reshape(128, _IN)
    )


def _make_bacc_cls():
    import concourse.bacc as bacc
    import concourse.mybir as mybir
    from concourse.hw_specs import get_activation_tables
    import bass_rust as _bass_rust

    class PinnedActBacc(bacc.Bacc):
        """Force all activations onto one table set so the kernel pays a
        single ACT table load instead of one per ln<->exp transition."""

        def insert_act_table_loads(self):
            has_activation = any(
                isinstance(i, mybir.InstActivation)
                for b in self.main_func.blocks
                for i in b.instructions
            )
            if not has_activation:
                return
            tables = list(get_activation_tables(self.m.arch).items())
            names = [t[0] for t in tables]
            if _PIN_ACT_SET not in names:
                _bass_rust.insert_act_table_loads(self, tables)
                return
            canon = names.index(_PIN_ACT_SET)
            keep = [tables[canon]]
            _bass_rust.insert_act_table_loads(self, keep)
            # the pass writes positional ids into the filtered list; remap
            # to the canonical act_info.json index walrus expects. Also
            # relocate the load to just before the first InstActivation so
            # ACT-issued DMAs aren't stuck behind the 1.3us table load.
            for b in self.main_func.blocks:
                loads = [
                    i
                    for i in b.instructions
                    if isinstance(i, mybir.InstLoadActFuncSet)
                ]
                if not loads:
                    continue
                for i in loads:
                    i.act_func_set_id = canon
                assert len(loads) == 1
                ld = loads[0]
                b.instructions.remove(ld)
                idx = next(
                    j
                    for j, i in enumerate(b.instructions)
                    if isinstance(i, mybir.InstActivation)
                )
                b.instructions.insert(idx, ld)

    return PinnedActBacc


def _build(
    loop_n=None,
    nwarm=2,
    warm_w=384,
    xch=1,
    wmch=1,
    ech=2,
    sout=True,
    order=(("xt", "sp"), ("wn", "act"), ("wm", "sp")),
):
    import concourse.mybir as mybir
    import concourse.tile as tile
    from concourse.bass import ts
    from concourse.bass_utils import run_bass_kernel_spmd

    dt = mybir.dt.float32
    mdt = getattr(mybir.dt, _MMDT)
    AF = mybir.ActivationFunctionType
    OP = mybir.AluOpType

    nc = _make_bacc_cls()(
        "TRN2",
        target_bir_lowering=False,
        debug=False,
        enable_asserts=False,
        num_devices=_NC,
    )
    # wn carries a 9th K-chunk whose partition-rows 0-1 hold the bias
    # hi/lo bf16 rows (rest zeros) — the bias rides the weight DMA.
    xT = nc.dram_tensor("xT", [128, _IN], mdt, kind="ExternalInput").ap()
    wn = nc.dram_tensor("wn", [128, _IN + _OSH], mdt, kind="ExternalInput").ap()
    wm = nc.dram_tensor("wm", [128, _IN], mdt, kind="ExternalInput").ap()
    out = nc.dram_tensor("out", [_B, _OSH], dt, kind="ExternalOutput").ap()

    def body(tc, io, ep, ps):
        # SBUF tiles for the streams
        XW = _IN // xch
        WMW = _IN // wmch
        xt = [io.tile([128, XW], mdt, name=f"xt{h}") for h in range(xch)]
        wnt = io.tile([128, _IN + _OSH], mdt, name="wnt")
        wmt = [io.tile([128, WMW], mdt, name=f"wmt{h}") for h in range(wmch)]

        # DMA issue split across both HWDGE rings; ACT's issue comes
        # before its table load in program order so the load doesn't
        # delay the stream. order entries: (name, engine)
        for name, eng in order:
            e = nc.sync if eng == "sp" else nc.scalar
            if name == "xt":
                for h in range(xch):
                    e.dma_start(
                        out=xt[h][:], in_=xT[:, h * XW : (h + 1) * XW]
                    )
            elif name == "wn":
                e.dma_start(out=wnt[:], in_=wn[:])
            elif name == "wm":
                for h in range(wmch):
                    e.dma_start(
                        out=wmt[h][:], in_=wm[:, h * WMW : (h + 1) * WMW]
                    )

        # ACT table warm: pulls the single natural_log_exp set load early.
        warm0 = ep.tile([1, 1], dt)
        nc.gpsimd.memset(warm0[:], 1.0)
        warm1 = ep.tile([1, 1], dt)
        nc.scalar.activation(warm1[:], warm0[:], AF.Ln)

        # constants off the critical engines
        wsrc = io.tile([2, warm_w], mybir.dt.bfloat16)
        nc.gpsimd.memset(wsrc[:], 0.0)
        ones = io.tile([2, _B], mybir.dt.bfloat16)
        nc.gpsimd.memset(ones[:], 1.0)
        # eps doubles as the u >= -42.5 clamp: ln(3.36e-19) = -42.5, which
        # keeps t = n*u >= -85 so exp(-t) stays finite (n < 2)
        eps = ep.tile([128, 1], dt)
        nc.gpsimd.memset(eps[:], 3.36e-19)

        # Output path: the final SBUF->DRAM store goes through SWDGE with
        # descriptors prepared EARLY on the idle Pool engine; when res is
        # ready, a cheap trigger_dma fires the transfer. This skips the
        # ~1.3us of HWDGE generation + DGE delay that a plain dma_start
        # would put after res on the critical path. The scatter ADDS into
        # the pre-zeroed output buffer, which equals a plain store.
        res_t = []
        if sout:
            idxs = io.tile([128, 8], mybir.dt.int16)
            nc.gpsimd.iota(idxs[:], pattern=[[16, 8]],
                           channel_multiplier=1)
            # only partitions 0-15 are read; clamp the rest below the
            # dst-row bound the executor asserts on
            nc.gpsimd.tensor_scalar_min(idxs[:], idxs[:], 127)
            EW0 = _OSH // ech
            for h in range(ech):
                res_t.append(ep.tile([_B, 1, EW0], dt, name=f"res{h}"))
                nc.gpsimd.memset(res_t[h][:], 0.0)
            sem = nc.alloc_semaphore("res_dma")
            for h in range(ech):
                nc.gpsimd.dma_scatter_add(
                    out[:, h * EW0 : (h + 1) * EW0], res_t[h][:],
                    idxs[:], 128, 128, EW0,
                    elem_step=_OSH, prepare_only=True, sem=sem,
                )

        # PE warmup: dummy matmuls starting at t~0 lift the HAM clock gate
        # to full rate before the real matmuls arrive.
        wp = ps.tile([128, warm_w], dt)
        for k in range(nwarm):
            nc.tensor.matmul(
                wp[:], wsrc[:, 0:128], wsrc[:],
                start=(k == 0), stop=(k == nwarm - 1),
            )

        # z_non group: bias rows first (off the tail), then 8 K-chunks
        zn = ps.tile([_B, _OSH], dt)
        nc.tensor.matmul(
            zn[:], ones, wnt[0:2, ts(_KT, _OSH)], start=True, stop=False
        )
        for j in range(_KT):
            xh, xj = divmod(j * 128, XW)
            nc.tensor.matmul(
                zn[:], xt[xh][:, xj : xj + 128], wnt[:, ts(j, _OSH)],
                start=False, stop=(j == _KT - 1),
            )

        # state sigmoid chain starts now, overlapping the z_nmda matmuls:
        # s = sigmoid(zn) = 1/(1+exp(-zn))
        e0 = ep.tile([_B, _OSH], dt)
        nc.scalar.activation(e0[:], zn[:], AF.Exp, scale=-1.0)
        d0 = ep.tile([_B, _OSH], dt)
        nc.vector.tensor_scalar_add(d0[:], e0[:], 1.0)
        s = ep.tile([_B, _OSH], dt)
        nc.vector.reciprocal_approx_fast(s[:], d0[:])

        # z_nmda: ech column-chunks, each its own PSUM group + epilogue
        # chain + out DMA, so chunk epilogues pipeline across ACT/DVE and
        # early chunks' output DMAs overlap later chunks' compute.
        EW = _OSH // ech
        zm = [ps.tile([_B, EW], dt, name=f"zm{h}") for h in range(ech)]
        for h in range(ech):
            for j in range(_KT):
                xh, xj = divmod(j * 128, XW)
                wh, wj = divmod(j * 128, WMW)
                nc.tensor.matmul(
                    zm[h][:],
                    xt[xh][:, xj : xj + 128],
                    wmt[wh][:, wj + h * EW : wj + h * EW + EW],
                    start=(j == 0), stop=(j == _KT - 1),
                )

        for h in range(ech):
            c = slice(h * EW, (h + 1) * EW)
            # u = ln(2*relu(zm) + eps); relu on ACT (PSUM in AND out =
            # cheaper ACT access), ln back-to-back on ACT
            ca2 = ps.tile([_B, EW], dt, name=f"ca2{h}")
            nc.scalar.activation(ca2[:], zm[h][:], AF.Relu, scale=2.0)
            u = ep.tile([_B, EW], dt, name=f"u{h}")
            nc.scalar.activation(u[:], ca2[:], AF.Ln, bias=eps[:])
            # t = (s+1)*u = n_modif * u
            t = ep.tile([_B, EW], dt, name=f"t{h}")
            nc.vector.scalar_tensor_tensor(
                t[:], s[:, c], 1.0, u[:], OP.add, OP.mult
            )
            # y = sigmoid(t)
            e1 = ep.tile([_B, EW], dt, name=f"e1{h}")
            nc.scalar.activation(e1[:], t[:], AF.Exp, scale=-1.0)
            d1 = ep.tile([_B, EW], dt, name=f"d1{h}")
            nc.vector.tensor_scalar_add(d1[:], e1[:], 1.0)
            y = ep.tile([_B, EW], dt, name=f"y{h}")
            nc.vector.reciprocal_approx_fast(y[:], d1[:])
            # out = y + (s - 1)
            if sout:
                res = res_t[h]
                rv = res[:, 0]
            else:
                res = ep.tile([_B, EW], dt, name=f"res{h}")
                rv = res[:]
            nc.vector.scalar_tensor_tensor(
                rv, y[:], -1.0, s[:, c], OP.add, OP.add
            )
            if sout:
                if h == ech - 1:
                    nc.gpsimd.trigger_dma(count=ech)
            else:
                nc.sync.dma_start(out=out[:, c], in_=res[:])

    with tile.TileContext(nc) as tc:
        with (
            tc.tile_pool(name="io", bufs=1) as io,
            tc.tile_pool(name="ep", bufs=1) as ep,
            tc.tile_pool(name="ps", bufs=1, space="PSUM") as ps,
        ):
            if loop_n is None:
                body(tc, io, ep, ps)
            else:
                with tc.For_i(0, loop_n, 1):
                    body(tc, io, ep, ps)

    nc.compile()
    return nc, run_bass_kernel_spmd


def _prep_in_maps(inputs, W_nmda, W_non, b_non):
    import ml_dtypes

    npdt = ml_dtypes.bfloat16

    x = np.ascontiguousarray(np.asarray(inputs, dtype=np.float32))
    Wn = np.asarray(W_non, dtype=np.float32)
    Wm = np.asarray(W_nmda, dtype=np.float32).copy()
    Wm[:, 0] *= 0.5
    Wm[:, -1] *= 0.5
    b = np.asarray(b_non, dtype=np.float32)
    # bias applied as two bf16 K-rows: bh + bl reproduces b to ~2^-17
    bh = b.astype(ml_dtypes.bfloat16).astype(np.float32)
    bl = b - bh

    xr = _rearr(x).astype(npdt)
    in_maps = []
    for c in range(_NC):
        sl = slice(c * _OSH, (c + 1) * _OSH)
        wnr = np.zeros((128, _IN + _OSH), np.float32)
        wnr[:, : _IN] = _rearr(Wn[sl])
        wnr[0, _IN:] = bh[sl]
        wnr[1, _IN:] = bl[sl]
        wmr = _rearr(Wm[sl]).astype(npdt)
        in_maps.append(
            {"xT": xr, "wn": wnr.astype(npdt), "wm": wmr}
        )
    return in_maps


def kernel(inputs, W_nmda, W_non, b_non):
    if "nc" not in _state:
        _state["nc"], _state["run"] = _build()
    nc, run = _state["nc"], _state["run"]
    in_maps = _prep_in_maps(inputs, W_nmda, W_non, b_non)
    res = run(nc, in_maps, list(range(_NC)))
    outs = res.results
    return np.concatenate([outs[c]["out"] for c in range(_NC)], axis=1)


# revision 26
# speedup vs baseline: 2.3448x; 2.3448x over previous
"""Trainium2 Bass kernel for DendriticFullyConnected.

Math (B=128, IN=OUT=1024):
    state = sigmoid(x @ W_non.T + b_non) - 1
    syn   = x[:,None,:] * W_nmda[None,:,:]            # [B,O,I]
    clus  = 0.5*(syn[...,:-1] + syn[...,1:])          # conv [0.5,0.5]
    ca    = relu(clus.sum(-1))
    n     = 2 + state;  kd = 0.5**n;  xn = ca**n
    out   = xn/(kd+xn) + state

Key algebraic collapse: the conv+sum equals a plain dot product against
W_nmda with its first and last columns halved:
    clus.sum(-1)[b,o] = sum_i x[b,i]*Wm[o,i],  Wm = W_nmda w/ cols {0,-1} * 0.5
so the [B,O,I] tensor never exists - the whole module is two matmuls:
    z_non = x @ W_non.T + b_non ;  z_nmda = x @ Wm.T
and the Hill fraction is a sigmoid in log space:
    xn/(kd+xn) = sigmoid(n * (ln(ca) + ln 2))

Sharding: OUT split 8 ways (128 outputs/core), x replicated. Per-core HBM
traffic ~832KB; every weight byte is read exactly once across the chip.

Device-side design (v2):
- Host prepares all layouts; zero on-device transposes. Contraction index
  lands on SBUF partitions for both operands.
- z_non and z_nmda accumulate in SEPARATE PSUM groups with SEPARATE weight
  streams (W_non first). The state sigmoid chain (exp/add/recip) runs
  while the z_nmda matmuls + W_nmda DMA are still in flight.
- DMA issue split across both HWDGE rings: SP issues x + W_nmda,
  ACT issues W_non + bias (before its table load in program order).
- Matmul inputs are bf16 (f32 PSUM accumulate): memory-bound, so halving
  HBM traffic wins; measured rel-err ~2e-3 vs f32 reference (gate 2e-2).
  The b_non bias is applied as two bf16 K-rows (hi+lo split, exact to
  ~2^-17) via lhsT=ones, rhs=[b_hi],[b_lo] appended to the z_non group.
- Dummy matmuls at t~0 warm the PE HAM clock gate before the real
  matmuls arrive; memsets ride on Pool (idle) not DVE.
- Activation table pinned to natural_log_exp_and_others (exp+ln+relu) so
  the kernel pays a single ACT table load, pulled to t=0 by a dummy Ln.
"""

import numpy as np

_B, _IN, _OUT, _NC = 128, 1024, 1024, 8
_OSH = _OUT // _NC  # 128 outputs per core
_KT = _IN // 128    # 8 contraction chunks
_MMDT = "bfloat16"  # matmul input dtype

_PIN_ACT_SET = "natural_log_exp_and_others"

_state = {}


def _rearr(m):
    # [128 rows, 1024 cols] -> out[p, j*128 + r] = m[r, j*128 + p]
    # per 128-column chunk j: transpose so the contraction index is the
    # partition dim and the row index is the free dim.
    return np.ascontiguousarray(
        m.reshape(128, _KT, 128).transpose(2, 1, 0).reshape(128, _IN)
    )


def _make_bacc_cls():
    import concourse.bacc as bacc
    import concourse.mybir as mybir
    from concourse.hw_specs import get_activation_tables
    import bass_rust as _bass_rust

    class PinnedActBacc(bacc.Bacc):
        """Force all activations onto one table set so the kernel pays a
        single ACT table load instead of one per ln<->exp transition."""

        def insert_act_table_loads(self):
            has_activation = any(
                isinstance(i, mybir.InstActivation)
                for b in self.main_func.blocks
                for i in b.instructions
            )
            if not has_activation:
                return
            tables = list(get_activation_tables(self.m.arch).items())
            names = [t[0] for t in tables]
            if _PIN_ACT_SET not in names:
                _bass_rust.insert_act_table_loads(self, tables)
                return
            canon = names.index(_PIN_ACT_SET)
            keep = [tables[canon]]
            _bass_rust.insert_act_table_loads(self, keep)
            # the pass writes positional ids into the filtered list; remap
            # to the canonical act_info.json index walrus expects. Also
            # relocate the load to just before the first InstActivation so
            # ACT-issued DMAs aren't stuck behind the 1.3us table load.
            for b in self.main_func.blocks:
                loads = [
                    i
                    for i in b.instructions
                    if isinstance(i, mybir.InstLoadActFuncSet)
                ]
                if not loads:
                    continue
                for i in loads:
                    i.act_func_set_id = canon
                assert len(loads) == 1
                ld = loads[0]
                b.instructions.remove(ld)
                idx = next(
                    j
                    for j, i in enumerate(b.instructions)
                    if isinstance(i, mybir.InstActivation)
                )
                b.instructions.insert(idx, ld)

    return PinnedActBacc


def _build(
    loop_n=None,
    nwarm=2,
    warm_w=384,
    xch=1,
    wmch=1,
    ech=1,
    sout=False,
    order=(("xt", "sp"), ("wn", "act"), ("wm", "sp")),
):
    import concourse.mybir as mybir
    import concourse.tile as tile
    from concourse.bass import ts
    from concourse.bass_utils import run_bass_kernel_spmd

    dt = mybir.dt.float32
    mdt = getattr(mybir.dt, _MMDT)
    AF = mybir.ActivationFunctionType
    OP = mybir.AluOpType

    nc = _make_bacc_cls()(
        "TRN2",
        target_bir_lowering=False,
        debug=False,
        enable_asserts=False,
        num_devices=_NC,
    )
    # wn carries a 9th K-chunk whose partition-rows 0-1 hold the bias
    # hi/lo bf16 rows (rest zeros) — the bias rides the weight DMA.
    xT = nc.dram_tensor("xT", [128, _IN], mdt, kind="ExternalInput").ap()
    wn = nc.dram_tensor("wn", [128, _IN + _OSH], mdt, kind="ExternalInput").ap()
    wm = nc.dram_tensor("wm", [128, _IN], mdt, kind="ExternalInput").ap()
    out = nc.dram_tensor("out", [_B, _OSH], dt, kind="ExternalOutput").ap()

    def body(tc, io, ep, ps):
        # SBUF tiles for the streams
        XW = _IN // xch
        WMW = _IN // wmch
        xt = [io.tile([128, XW], mdt, name=f"xt{h}") for h in range(xch)]
        wnt = io.tile([128, _IN + _OSH], mdt, name="wnt")
        wmt = [io.tile([128, WMW], mdt, name=f"wmt{h}") for h in range(wmch)]

        # DMA issue split across both HWDGE rings; ACT's issue comes
        # before its table load in program order so the load doesn't
        # delay the stream. order entries: (name, engine)
        for name, eng in order:
            e = nc.sync if eng == "sp" else nc.scalar
            if name == "xt":
                for h in range(xch):
                    e.dma_start(
                        out=xt[h][:], in_=xT[:, h * XW : (h + 1) * XW]
                    )
            elif name == "wn":
                e.dma_start(out=wnt[:], in_=wn[:])
            elif name == "wm":
                for h in range(wmch):
                    e.dma_start(
                        out=wmt[h][:], in_=wm[:, h * WMW : (h + 1) * WMW]
                    )

        # ACT table warm: pulls the single natural_log_exp set load early.
        warm0 = ep.tile([1, 1], dt)
        nc.vector.memset(warm0[:], 1.0)
        warm1 = ep.tile([1, 1], dt)
        nc.scalar.activation(warm1[:], warm0[:], AF.Ln)

        # constants off the critical engines
        wsrc = io.tile([2, warm_w], mybir.dt.bfloat16)
        nc.vector.memset(wsrc[:], 0.0)
        ones = io.tile([2, _B], mybir.dt.bfloat16)
        nc.vector.memset(ones[:], 1.0)
        # eps doubles as the u >= -42.5 clamp: ln(3.36e-19) = -42.5, which
        # keeps t = n*u >= -85 so exp(-t) stays finite (n < 2)
        eps = ep.tile([128, 1], dt)
        nc.vector.memset(eps[:], 3.36e-19)

        # Output path: the final SBUF->DRAM store goes through SWDGE with
        # descriptors prepared EARLY on the idle Pool engine; when res is
        # ready, a cheap trigger_dma fires the transfer. This skips the
        # ~1.3us of HWDGE generation + DGE delay that a plain dma_start
        # would put after res on the critical path. The scatter ADDS into
        # the pre-zeroed output buffer, which equals a plain store.
        res_t = []
        if sout:
            idxs = io.tile([128, 8], mybir.dt.int16)
            nc.gpsimd.iota(idxs[:], pattern=[[16, 8]],
                           channel_multiplier=1)
            # only partitions 0-15 are read; clamp the rest below the
            # dst-row bound the executor asserts on
            nc.gpsimd.tensor_scalar_min(idxs[:], idxs[:], 127)
            EW0 = _OSH // ech
            for h in range(ech):
                res_t.append(ep.tile([_B, 1, EW0], dt, name=f"res{h}"))
                nc.gpsimd.memset(res_t[h][:], 0.0)
            sem = nc.alloc_semaphore("res_dma")
            for h in range(ech):
                nc.gpsimd.dma_scatter_add(
                    out[:, h * EW0 : (h + 1) * EW0], res_t[h][:],
                    idxs[:], 128, 128, EW0,
                    elem_step=_OSH, prepare_only=True, sem=sem,
                )

        # PE warmup: dummy matmuls starting at t~0 lift the HAM clock gate
        # to full rate before the real matmuls arrive.
        wp = ps.tile([128, warm_w], dt)
        for k in range(nwarm):
            nc.tensor.matmul(
                wp[:], wsrc[:, 0:128], wsrc[:],
                start=(k == 0), stop=(k == nwarm - 1),
            )

        # z_non group: bias rows first (off the tail), then 8 K-chunks
        zn = ps.tile([_B, _OSH], dt)
        nc.tensor.matmul(
            zn[:], ones, wnt[0:2, ts(_KT, _OSH)], start=True, stop=False
        )
        for j in range(_KT):
            xh, xj = divmod(j * 128, XW)
            nc.tensor.matmul(
                zn[:], xt[xh][:, xj : xj + 128], wnt[:, ts(j, _OSH)],
                start=False, stop=(j == _KT - 1),
            )

        # state sigmoid chain starts now, overlapping the z_nmda matmuls:
        # s = sigmoid(zn) = 1/(1+exp(-zn))
        e0 = ep.tile([_B, _OSH], dt)
        nc.scalar.activation(e0[:], zn[:], AF.Exp, scale=-1.0)
        d0 = ep.tile([_B, _OSH], dt)
        nc.vector.tensor_scalar_add(d0[:], e0[:], 1.0)
        s = ep.tile([_B, _OSH], dt)
        nc.vector.reciprocal_approx_fast(s[:], d0[:])

        # z_nmda: ech column-chunks, each its own PSUM group + epilogue
        # chain + out DMA, so chunk epilogues pipeline across ACT/DVE and
        # early chunks' output DMAs overlap later chunks' compute.
        EW = _OSH // ech
        zm = [ps.tile([_B, EW], dt, name=f"zm{h}") for h in range(ech)]
        for h in range(ech):
            for j in range(_KT):
                xh, xj = divmod(j * 128, XW)
                wh, wj = divmod(j * 128, WMW)
                nc.tensor.matmul(
                    zm[h][:],
                    xt[xh][:, xj : xj + 128],
                    wmt[wh][:, wj + h * EW : wj + h * EW + EW],
                    start=(j == 0), stop=(j == _KT - 1),
                )

        for h in range(ech):
            c = slice(h * EW, (h + 1) * EW)
            # u = ln(2*relu(zm) + eps); relu on ACT (PSUM in AND out =
            # cheaper ACT access), ln back-to-back on ACT
            ca2 = ps.tile([_B, EW], dt, name=f"ca2{h}")
            nc.scalar.activation(ca2[:], zm[h][:], AF.Relu, scale=2.0)
            u = ep.tile([_B, EW], dt, name=f"u{h}")
            nc.scalar.activation(u[:], ca2[:], AF.Ln, bias=eps[:])
            # t = (s+1)*u = n_modif * u
            t = ep.tile([_B, EW], dt, name=f"t{h}")
            nc.vector.scalar_tensor_tensor(
                t[:], s[:, c], 1.0, u[:], OP.add, OP.mult
            )
            # y = sigmoid(t)
            e1 = ep.tile([_B, EW], dt, name=f"e1{h}")
            nc.scalar.activation(e1[:], t[:], AF.Exp, scale=-1.0)
            d1 = ep.tile([_B, EW], dt, name=f"d1{h}")
            nc.vector.tensor_scalar_add(d1[:], e1[:], 1.0)
            y = ep.tile([_B, EW], dt, name=f"y{h}")
            nc.vector.reciprocal_approx_fast(y[:], d1[:])
            # out = y + (s - 1)
            if sout:
                res = res_t[h]
                rv = res[:, 0]
            else:
                res = ep.tile([_B, EW], dt, name=f"res{h}")
                rv = res[:]
            nc.vector.scalar_tensor_tensor(
                rv, y[:], -1.0, s[:, c], OP.add, OP.add
            )
            if sout:
                if h == ech - 1:
                    nc.gpsimd.trigger_dma(count=ech)
            else:
                nc.sync.dma_start(out=out[:, c], in_=res[:])

    with tile.TileContext(nc) as tc:
        with (
            tc.tile_pool(name="io", bufs=1) as io,
            tc.tile_pool(name="ep", bufs=1) as ep,
            tc.tile_pool(name="ps", bufs=1, space="PSUM") as ps,
        ):
            if loop_n is None:
                body(tc, io, ep, ps)
            else:
                with tc.For_i(0, loop_n, 1):
                    body(tc, io, ep, ps)

    nc.compile()
    return nc, run_bass_kernel_spmd


def _prep_in_maps(inputs, W_nmda, W_non, b_non):
    import ml_dtypes

    npdt = ml_dtypes.bfloat16

    x = np.ascontiguousarray(np.asarray(inputs, dtype=np.float32))
    Wn = np.asarray(W_non, dtype=np.float32)
    Wm = np.asarray(W_nmda, dtype=np.float32).copy()
    Wm[:, 0] *= 0.5
    Wm[:, -1] *= 0.5
    b = np.asarray(b_non, dtype=np.float32)
    # bias applied as two bf16 K-rows: bh + bl reproduces b to ~2^-17
    bh = b.astype(ml_dtypes.bfloat16).astype(np.float32)
    bl = b - bh

    xr = _rearr(x).astype(npdt)
    in_maps = []
    for c in range(_NC):
        sl = slice(c * _OSH, (c + 1) * _OSH)
        wnr = np.zeros((128, _IN + _OSH), np.float32)
        wnr[:, : _IN] = _rearr(Wn[sl])
        wnr[0, _IN:] = bh[sl]
        wnr[1, _IN:] = bl[sl]
        wmr = _rearr(Wm[sl]).astype(npdt)
        in_maps.append(
            {"xT": xr, "wn": wnr.astype(npdt), "wm": wmr}
        )
    return in_maps


def kernel(inputs, W_nmda, W_non, b_non):
    if "nc" not in _state:
        _state["nc"], _state["run"] = _build()
    nc, run = _state["nc"], _state["run"]
    in_maps = _prep_in_maps(inputs, W_nmda, W_non, b_non)
    res = run(nc, in_maps, list(range(_NC)))
    outs = res.results
    return np.concatenate([outs[c]["out"] for c in range(_NC)], axis=1)
